# revision 21
# baseline (speedup 1.0000x reference)
"""EncNet vq_codebook kernel for 8 Trainium2 NeuronCores.

Math (per reference):
  xs = x[:, :, 0, :].T                         # (b, s, c)
  d2[s,k]   = x2[s] - 2*cross[s,k] + cw2[k]
  a         = softmax_k(sm[k] * d2)
  e[b,k,c]  = sum_s a*xs - (sum_s a)*cw[k,c]
  BN over (b,c) (training stats), relu, mean over k, fc, sigmoid
  out = x * scale[b,c]

Distribution: data-parallel over batch (2 batches per core); BN batch
stats all-gathered across the 8 cores via direct remote SBUF writes
(XOR-slot allgather: core c's k-th transfer goes to peer c^k writing
static slot k; every receiver just sums all 8 slots).

On-core layout: s-chunks of 128 land on PSUM partitions.  x is loaded
from HBM with an f32->bf16 cast riding the SWDGE DMA, so the resident
copy is bf16 (8 MiB) and no engine spends time casting.  With an
x-chunk (c=128, s=128) as PE weights:
  - rhs = I (is_transpose)   -> xT chunk (s, c) in bf16 PSUM (1 bank)
  - rhs = -2*sm_k*cw[k,c]    -> -2*sm_k*cross[s,k]
and with x^2 (fp16) as weights:
  - rhs = smhi/smlo (fp16)   -> sm_k * x2[s]        (exact hi+lo split of sm)
so PSUM accumulates L[s,k] = sm_k*(x2[s] - 2cross[s,k]).  The constant
sm_k*cw2_k logit term is dropped: it is a pure per-k scale exp(sm*cw2)
on araw (modulo a <=0.5% per-s reweighting through Z) and BatchNorm
over (b,c) is exactly invariant to per-k scaling of e.  Logits are
<= ~0.006 by construction so exp without max-subtraction is safe.

Per softmax group (1024 s): exp on ACT, the k-sum + reciprocal + the
xT psum->sbuf copy (with 1/Z folded in) on DVE, x^2 on ACT.  The per-s
1/Z lands (bf16) in column C of the xtn tile so one 129-col matmul per
subchunk accumulates both e1 and asum.

The cross-core waits (gpsimd barrier before the trigger, vector wait on
the arrival semaphore) are emitted after the TileContext and spliced
into the scheduled stream: the tile scheduler's single-core simulation
cannot model remote semaphore increments and would deadlock.
"""

import sys

import numpy as np

try:
    import concourse.bass as bass  # noqa: F401
except ImportError:
    sys.path.insert(0, "/opt/trn_rl_repo")

import concourse.bacc as bacc
import concourse.bass as bass
import concourse.mybir as mybir
import concourse.tile as tile
from concourse.bass_utils import run_bass_kernel_spmd
from concourse._compat import get_trn_type
from ml_dtypes import bfloat16
float16 = np.float16

F32 = mybir.dt.float32
BF16 = mybir.dt.bfloat16
FP16 = mybir.dt.float16
ALU = mybir.AluOpType
ACTF = mybir.ActivationFunctionType

N_CORES = 8
B, C, SEQ, K = 16, 128, 16384, 64
B_LOC = B // N_CORES           # 2 batches per core
BIG = 2048                     # phase-1 DMA chunk (free dim)
OBIG = 4096                    # phase-2 output chunk
GRP = 1024                     # softmax group: 8 subchunks share PSUM banks
SUB = 128                      # s-subchunk = PSUM partition dim
BN_EPS = 1e-5

USE_REMOTE_AR = True           # manual SBUF allgather vs gpsimd collective


def build_program(seq=SEQ, b_loc=B_LOC, n_cores=N_CORES, big=BIG):
    n_big = seq // big
    n_grp = big // GRP
    n_sub = GRP // SUB         # 8
    CP = C + 1                 # xtn row: [xT cols | rz]

    nc = bacc.Bacc(
        get_trn_type() or "TRN2",
        target_bir_lowering=False,
        debug=False,
        num_devices=n_cores,
    )

    x_ap = nc.dram_tensor("x", [b_loc, C, seq], F32, kind="ExternalInput").ap()
    out_ap = nc.dram_tensor("out", [b_loc, C, seq], F32, kind="ExternalOutput").ap()

    def const_in(name, shape, dt):
        return nc.dram_tensor(name, shape, dt, kind="ExternalInput").ap()

    identcw_d = const_in("identcw_bf", [C, C + K], BF16)
    smhl_d = const_in("smhl_fp16", [C, 2 * K], FP16)
    cw_rows_d = const_in("cw_rows", [K, C], F32)
    gamma_d = const_in("gamma_col", [K, 1], F32)
    beta_d = const_in("beta_col", [K, 1], F32)
    fc_wt_d = const_in("fc_wt", [C, C], F32)
    fc_b_d = const_in("fc_b_col", [C, 1], F32)
    invk_d = const_in("invk_col", [K, 1], F32)

    with tile.TileContext(nc) as tc:
        with (
            tc.tile_pool(name="consts", bufs=1) as cpool,
            tc.tile_pool(name="xbf", bufs=2) as xbfp,
            tc.tile_pool(name="xsq", bufs=2) as xsqp,
            tc.tile_pool(name="soft", bufs=6) as softp,
            tc.tile_pool(name="cols", bufs=8) as colp,
            tc.tile_pool(name="xt", bufs=6) as xtp,
            tc.tile_pool(name="etail", bufs=4) as etailp,
            tc.tile_pool(name="eloc", bufs=2) as elocp,
            tc.tile_pool(name="scales", bufs=2) as scalep,
            tc.tile_pool(name="og", bufs=2) as ogp,
            tc.tile_pool(name="ps_xt", bufs=3, space="PSUM") as ps_xt,
            tc.tile_pool(name="ps_L", bufs=3, space="PSUM") as ps_L,
            tc.tile_pool(name="ps_e", bufs=1, space="PSUM") as ps_e,
            tc.tile_pool(name="ps_tail", bufs=1, space="PSUM") as ps_tail,
            tc.tile_pool(name="dram", bufs=2, space="DRAM") as dram,
        ):
            # ---- load constants into SBUF once ----
            def load_const(dram_ap, shape, dt):
                t = cpool.tile(shape, dt, tag=dram_ap.tensor.name)
                nc.sync.dma_start(out=t[:], in_=dram_ap[:])
                return t

            identcw = load_const(identcw_d, [C, C + K], BF16)
            ident = identcw[:, 0:C]
            cwt_sm = identcw[:, C : C + K]
            smhl = load_const(smhl_d, [C, 2 * K], FP16)
            smhi = smhl[:, 0:K]
            smlo = smhl[:, K : 2 * K]
            cw_rows = load_const(cw_rows_d, [K, C], F32)
            gamma = load_const(gamma_d, [K, 1], F32)
            beta = load_const(beta_d, [K, 1], F32)
            fc_wt = load_const(fc_wt_d, [C, C], F32)
            fc_b = load_const(fc_b_d, [C, 1], F32)
            invk = load_const(invk_d, [K, 1], F32)

            if USE_REMOTE_AR:
                stats = cpool.tile([128, 2], F32, tag="stats_t")
                gath = cpool.tile([128, 2 * n_cores], F32, tag="gath")
                rsem = nc.alloc_semaphore("gath_rsem")
                lsem = nc.alloc_semaphore("gath_lsem")

            # ---- phase 1: per-batch aggregation e1|asum ----
            # x is cast to bf16 during the load and stays resident in SBUF
            # (reused by the phase-2 scale), so HBM traffic is one read +
            # one write of x.
            e_sbs = []
            xres = []
            for b in range(b_loc):
                e_ps = ps_e.tile([K, CP], F32)
                e_first = True
                xbf = xbfp.tile([C, seq], BF16, tag=f"xbf{b}")
                xres.append(xbf)
                for j in range(n_big):
                    jsl = slice(j * big, (j + 1) * big)
                    # two 1024-col cast-DMAs per big chunk: finer arrival
                    # granularity so hoisted transposes stall less
                    for h in range(2):
                        hsl = slice(j * big + h * (big // 2),
                                    j * big + (h + 1) * (big // 2))
                        nc.gpsimd.dma_start(out=xbf[:, hsl], in_=x_ap[b, :, hsl])
                    xsq = xsqp.tile([C, big], FP16)
                    nc.scalar.square(xsq[:], xbf[:, jsl])
                    for g in range(n_grp):
                        g0 = j * big + g * GRP
                        # bf16 transpose output: 8 subtiles fit one PSUM bank
                        xt_ps = ps_xt.tile([SUB, n_sub * C], BF16)
                        L_ps = ps_L.tile([SUB, n_sub * K], F32)
                        # the constant sm_k*cw2_k logit term is dropped: it
                        # is a pure per-k scale exp(sm*cw2) on araw (up to a
                        # <=0.5% per-s reweighting via Z) and BatchNorm over
                        # (b,c) is exactly invariant to per-k scaling of e.
                        for i in range(n_sub):
                            sl = slice(g0 + i * SUB, g0 + (i + 1) * SUB)
                            sq_sl = slice(
                                g * GRP + i * SUB, g * GRP + (i + 1) * SUB
                            )
                            nc.tensor.matmul(
                                xt_ps[:, i * C : (i + 1) * C],
                                lhsT=xbf[:, sl], rhs=ident,
                                start=(i == 0), stop=(i == n_sub - 1),
                                is_transpose=True,
                                skip_group_check=True,
                            )
                            nc.tensor.matmul(
                                L_ps[:, i * K : (i + 1) * K],
                                lhsT=xbf[:, sl], rhs=cwt_sm,
                                start=True, stop=False,
                                skip_group_check=True,
                            )
                            nc.tensor.matmul(
                                L_ps[:, i * K : (i + 1) * K],
                                lhsT=xsq[:, sq_sl], rhs=smhi,
                                start=False, stop=False, skip_group_check=True,
                            )
                            nc.tensor.matmul(
                                L_ps[:, i * K : (i + 1) * K],
                                lhsT=xsq[:, sq_sl], rhs=smlo,
                                start=False, stop=(i == n_sub - 1),
                                skip_group_check=True,
                            )
                        # araw = exp(sm_k*d2) directly (cw2 already in L)
                        araw = softp.tile([SUB, n_sub * K], BF16, tag="araw")
                        nc.scalar.activation(araw[:], L_ps[:], ACTF.Exp)
                        zw = colp.tile([SUB, n_sub], F32, tag="zw")
                        nc.vector.tensor_reduce(
                            zw[:],
                            araw[:].rearrange("p (g k) -> p g k", g=n_sub),
                            mybir.AxisListType.X, ALU.add,
                        )
                        xtn = xtp.tile([SUB, n_sub * CP], BF16)
                        xtn_v = xtn[:].rearrange("p (g c) -> p g c", g=n_sub)
                        # 1/Z lands (bf16) in column C of each subchunk row
                        with nc.allow_low_precision(reason="1/Z in bf16 is plenty for 2e-2 rel tol"):
                            nc.vector.reciprocal(xtn_v[:, :, C : C + 1], zw[:])
                        nc.vector.tensor_tensor(
                            xtn_v[:, :, 0:C],
                            xt_ps[:].rearrange("p (g c) -> p g c", g=n_sub),
                            xtn_v[:, :, C : C + 1].broadcast_to(
                                [SUB, n_sub, C]
                            ),
                            ALU.mult,
                        )
                        for i in range(n_sub):
                            last = (
                                j == n_big - 1 and g == n_grp - 1
                                and i == n_sub - 1
                            )
                            nc.tensor.matmul(
                                e_ps[:, 0:CP],
                                lhsT=araw[:, i * K : (i + 1) * K],
                                rhs=xtn[:, i * CP : (i + 1) * CP],
                                start=e_first, stop=last, skip_group_check=True,
                            )
                            e_first = False
                e_sb = etailp.tile([K, CP], F32, tag="e_sb")
                nc.vector.tensor_copy(e_sb[:], e_ps[:])
                e_sbs.append(e_sb)

            # ---- local e + stats ----
            s1s, s2s, e_locs = [], [], []
            for b in range(b_loc):
                e_sb = e_sbs[b]
                easm = etailp.tile([K, C], F32, tag="easm")
                nc.vector.tensor_scalar(
                    out=easm[:], in0=cw_rows[:], scalar1=e_sb[:, C : C + 1],
                    scalar2=None, op0=ALU.mult,
                )
                e_loc = elocp.tile([K, C], F32)
                nc.vector.tensor_tensor(e_loc[:], e_sb[:, 0:C], easm[:], ALU.subtract)
                e_locs.append(e_loc)
                s1 = colp.tile([K, 1], F32, tag="s1")
                nc.vector.tensor_reduce(s1[:], e_loc[:], mybir.AxisListType.X, ALU.add)
                esq = etailp.tile([K, C], F32, tag="esq")
                nc.vector.tensor_tensor(esq[:], e_loc[:], e_loc[:], ALU.mult)
                s2 = colp.tile([K, 1], F32, tag="s2")
                nc.vector.tensor_reduce(s2[:], esq[:], mybir.AxisListType.X, ALU.add)
                s1s.append(s1)
                s2s.append(s2)

            if USE_REMOTE_AR:
                # ---- allgather BN stats via remote SBUF writes ----
                # stats: (128, 2) with s1|s2 on partitions 0:64, zeros below.
                nc.vector.memset(stats[64:128, :], 0.0)
                nc.vector.tensor_tensor(
                    stats[0:64, 0:1], s1s[0][:], s1s[1][:], ALU.add
                )
                nc.vector.tensor_tensor(
                    stats[0:64, 1:2], s2s[0][:], s2s[1][:], ALU.add
                )
                # slot 0 = self, local copy.  Slot k goes to peer
                # (me XOR k); receiver's slot k then holds the stats of
                # (receiver XOR k).  Sum over slots = allreduce.
                nc.vector.tensor_copy(gath[:, 0:2], stats[:])
                for k in range(1, n_cores):
                    rdests = [None] * 8
                    rdests[k] = (0, k)
                    nc.gpsimd.remote_dma_broadcast(
                        gath[:, 2 * k : 2 * k + 2],
                        stats[:],
                        remote_sem=rsem,
                        local_sem=lsem,
                        rdests=rdests,
                    )
                ar_trig = nc.gpsimd.trigger_dma(count=None)
                gst = etailp.tile([128, 2], F32, tag="gst")
                ar_red = nc.vector.tensor_reduce(
                    gst[:],
                    gath[:].rearrange("p (g t) -> p t g", g=n_cores),
                    mybir.AxisListType.X, ALU.add,
                )
                gk = gst[0:K, :]
            else:
                stats = etailp.tile([K, 2], F32, tag="stats")
                nc.vector.tensor_tensor(stats[:, 0:1], s1s[0][:], s1s[1][:], ALU.add)
                nc.vector.tensor_tensor(stats[:, 1:2], s2s[0][:], s2s[1][:], ALU.add)
                cc_in = dram.tile([K, 2], F32)
                cc_out = dram.tile([K, 2], F32)
                nc.sync.dma_start(out=cc_in[:], in_=stats[:])
                nc.gpsimd.collective_compute(
                    "AllReduce",
                    ALU.add,
                    replica_groups=[list(range(n_cores))],
                    ins=[cc_in.opt()],
                    outs=[cc_out.opt()],
                )
                gst = etailp.tile([K, 2], F32, tag="gst")
                nc.sync.dma_start(out=gst[:], in_=cc_out[:])
                gk = gst[:]

            # ---- BN affine + relu + mean_k + fc + sigmoid (tiny) ----
            n_tot = float(B * C)  # stats population: all b, all c
            mean = colp.tile([K, 1], F32, tag="mean")
            nc.vector.tensor_scalar(
                out=mean[:], in0=gk[:, 0:1], scalar1=1.0 / n_tot, scalar2=None,
                op0=ALU.mult,
            )
            ex2 = colp.tile([K, 1], F32, tag="ex2")
            nc.vector.tensor_scalar(
                out=ex2[:], in0=gk[:, 1:2], scalar1=1.0 / n_tot, scalar2=None,
                op0=ALU.mult,
            )
            msq = colp.tile([K, 1], F32, tag="msq")
            nc.vector.tensor_tensor(msq[:], mean[:], mean[:], ALU.mult)
            varep = colp.tile([K, 1], F32, tag="varep")
            nc.vector.tensor_tensor(varep[:], ex2[:], msq[:], ALU.subtract)
            nc.vector.tensor_scalar(
                out=varep[:], in0=varep[:], scalar1=BN_EPS, scalar2=None, op0=ALU.add
            )
            stdv = colp.tile([K, 1], F32, tag="stdv")
            nc.scalar.sqrt(stdv[:], varep[:])
            rstd = colp.tile([K, 1], F32, tag="rstd")
            nc.vector.reciprocal(rstd[:], stdv[:])
            psc = colp.tile([K, 1], F32, tag="psc")
            nc.vector.tensor_tensor(psc[:], gamma[:], rstd[:], ALU.mult)
            mps = colp.tile([K, 1], F32, tag="mps")
            nc.vector.tensor_tensor(mps[:], mean[:], psc[:], ALU.mult)
            pofs = colp.tile([K, 1], F32, tag="pofs")
            nc.vector.tensor_tensor(pofs[:], beta[:], mps[:], ALU.subtract)

            scale_cols = []
            for b in range(b_loc):
                reb = etailp.tile([K, C], F32, tag="reb")
                nc.scalar.activation(
                    reb[:], e_locs[b][:], ACTF.Relu, bias=pofs[:], scale=psc[:]
                )
                en_ps = ps_tail.tile([C, 1], F32, tag="tail")
                nc.tensor.matmul(
                    en_ps[:], lhsT=reb[:], rhs=invk[:], start=True, stop=True
                )
                en_sb = colp.tile([C, 1], F32, tag="en_sb")
                nc.vector.tensor_copy(en_sb[:], en_ps[:])
                fc_ps = ps_tail.tile([C, 1], F32, tag="tail")
                nc.tensor.matmul(
                    fc_ps[:], lhsT=fc_wt[:], rhs=en_sb[:], start=True, stop=True
                )
                sc = scalep.tile([C, 1], F32)
                nc.scalar.activation(sc[:], fc_ps[:], ACTF.Sigmoid, bias=fc_b[:])
                scale_cols.append(sc)

            # ---- phase 2: out = x * scale (bf16 x resident in SBUF) ----
            n_obig = seq // OBIG
            for b in range(b_loc):
                for j in range(n_obig):
                    jsl = slice(j * OBIG, (j + 1) * OBIG)
                    og = ogp.tile([C, OBIG], F32)
                    if (j + b) % 2 == 0:
                        nc.vector.tensor_scalar(
                            out=og[:], in0=xres[b][:, jsl],
                            scalar1=scale_cols[b][:], scalar2=None, op0=ALU.mult,
                        )
                        nc.sync.dma_start(out=out_ap[b, :, jsl], in_=og[:])
                    else:
                        nc.scalar.activation(
                            og[:], xres[b][:, jsl], ACTF.Copy,
                            scale=scale_cols[b][:],
                        )
                        nc.scalar.dma_start(out=out_ap[b, :, jsl], in_=og[:])

    if USE_REMOTE_AR:
        # The cross-core waits are emitted AFTER the TileContext so the
        # tile scheduler's single-core simulation (which cannot model
        # remote semaphore increments) never sees them, then spliced into
        # the scheduled instruction stream at the right spots.  Hardware
        # engines execute a block's instructions in list order per engine.
        barrier_w = nc.gpsimd.bir_kernel_barrier_wait([list(range(n_cores))])
        rsem_w = nc.vector.wait_ge(rsem, 2 * (n_cores - 1))
        _move_before(nc, barrier_w.ins, ar_trig.ins)
        _move_before(nc, rsem_w.ins, ar_red.ins)

    nc.compile()
    return nc


def _move_before(nc, ins, anchor):
    """Relocate `ins` to sit immediately before `anchor` in its block."""
    src_blk = tgt_blk = None
    for blk in nc.m.functions[0].blocks:
        names = [i.name for i in blk.instructions]
        if ins.name in names:
            src_blk = blk
        if anchor.name in names:
            tgt_blk = blk
    assert src_blk is not None and tgt_blk is not None, (ins.name, anchor.name)
    li = src_blk.instructions
    li.remove(li[[i.name for i in li].index(ins.name)])
    lt = tgt_blk.instructions
    lt.insert([i.name for i in lt].index(anchor.name), ins)


def make_const_inputs(codewords, smoothing, bn_weight, bn_bias, fc_w, fc_b):
    cw = np.asarray(codewords, np.float32)        # (K, C)
    sm = np.asarray(smoothing, np.float32)        # (K,)
    cw2 = (cw * cw).sum(1)                        # (K,)
    smhi = sm.astype(float16)
    smlo = (sm - smhi.astype(np.float32)).astype(float16)
    n_sub = GRP // SUB
    identcw = np.concatenate(
        [np.eye(C, dtype=bfloat16),
         (cw.T * (-2.0 * sm)[None, :]).astype(bfloat16)], axis=1)
    smhl = np.concatenate(
        [np.tile(smhi[None, :], (C, 1)), np.tile(smlo[None, :], (C, 1))], axis=1)
    consts = {
        "identcw_bf": identcw,
        "smhl_fp16": smhl,
        "cw_rows": np.ascontiguousarray(cw),
        "gamma_col": np.asarray(bn_weight, np.float32).reshape(K, 1),
        "beta_col": np.asarray(bn_bias, np.float32).reshape(K, 1),
        "fc_wt": np.ascontiguousarray(np.asarray(fc_w, np.float32).T),  # (C_in,C_out)
        "fc_b_col": np.asarray(fc_b, np.float32).reshape(C, 1),
        "invk_col": np.full((K, 1), 1.0 / K, np.float32),
    }
    return consts


_NC_CACHE = {}


def _get_program():
    key = (SEQ, B_LOC, N_CORES, BIG)
    if key not in _NC_CACHE:
        _NC_CACHE[key] = build_program(*key)
    return _NC_CACHE[key]


def _run(inputs, trace=False, trace_kwargs=None):
    x = np.asarray(inputs["x"], np.float32)
    assert x.shape == (B, C, 1, SEQ), x.shape
    xs = np.ascontiguousarray(x.reshape(B, C, SEQ))
    consts = make_const_inputs(
        inputs["codewords"], inputs["smoothing"], inputs["bn_weight"],
        inputs["bn_bias"], inputs["fc_w"], inputs["fc_b"],
    )
    in_maps = [
        {"x": np.ascontiguousarray(xs[i * B_LOC : (i + 1) * B_LOC]), **consts}
        for i in range(N_CORES)
    ]
    nc = _get_program()
    res = run_bass_kernel_spmd(
        nc, in_maps, core_ids=list(range(N_CORES)), trace=trace,
        **(trace_kwargs or {}),
    )
    out = np.concatenate([res.results[i]["out"] for i in range(N_CORES)], axis=0)
    return out.reshape(B, C, 1, SEQ).astype(np.float32), res


def kernel(**inputs):
    out, _ = _run(inputs)
    return out
